# revision 1
# baseline (speedup 1.0000x reference)
"""Trainium2 Bass kernel for nn_DiagonalKernelAverageLinear.

out[b,i] = relu( sum_c ring_c[b,i] * W[i,c] / (2i+1) + bias[i] )

ring decomposition (verified vs reference in fp64):
  ring_tl = RL + CL - d
  ring_tr = RT - RA + rev(CA)
  ring_bl = rev(RA) + CT - CA
  ring_br = rev(RT - RL + d + CT - CL)
with per-image length-512 vectors:
  RL[r] = sum_{w<=r} x[r,w]       RA[r] = sum_{w<=511-r} x[r,w]    RT = row sums
  CL[c] = sum_{h<=c} x[h,c]       CA[c] = sum_{h<=511-c} x[h,c]    CT = col sums
  d = diag(x),  rev(v)[i] = v[511-i]

Per-core (8 cores, 32 images each), per image [512,512] in SBUF as
X[128, 4t, 512w] (row-tile t), everything is built from:
  - B2[r',(t,s)]: 128-block row sums       (DVE segmented reduce)
  - RLp/RAp: masked reduce of diag/anti 128-blocks     (DVE fused TT-reduce)
  - CLp/CAp: same on PE-transposed diag/anti blocks
  - CT/CLcoarse/CAcoarse: PE matmuls with ones-column -> [3,512] PSUM,
    re-laid-out to [128,4] via small PE transposes
  - partition-reversal via PE matmul with a reversal permutation matrix
Final combine is batched over groups of G=8 images in [128, G*4] tiles.
"""

import dataclasses
import os
import numpy as np
from contextlib import ExitStack

import concourse.bass as bass
import concourse.mybir as mybir
import concourse.tile as tile
from concourse.bass_utils import run_bass_kernel_spmd

F32 = mybir.dt.float32
AL = mybir.AluOpType
AX = mybir.AxisListType

NCORES = 8
B_TOT, H, W = 256, 512, 512
BPC = B_TOT // NCORES          # 32 images per core
G = int(os.environ.get("KG", "8"))  # combine-group size
NG = BPC // G

POOL_OFFLOAD = False           # RAp/CAp masked ops on gpsimd instead of DVE
ABLATE = set(os.environ.get("ABLATE", "").split(",")) - {""}


def _emit_image_a(nc, tc, pools, consts, x_d, gi, g, gt, state):
    """Phase A: loads, row reduces, transposes, column matmuls, evacs."""
    (xpool, epool, spool, ppool, pcg) = pools
    ident, ones1 = consts["ident"], consts["ones1"]

    X = xpool.tile([128, 4, 512], F32, tag="X", name="X")
    if "xdma" in ABLATE:
        nc.sync.dma_start(X[:, :, 0:2], x_d[g, 0:128, 0:8].rearrange("p (t w) -> p t w", t=4))
    else:
        nxd = int(os.environ.get("NXDMA", "2"))
        per = 4 // nxd
        for h in range(nxd):
            nc.sync.dma_start(
                X[:, per * h:per * (h + 1), :],
                x_d[g, 128 * per * h:128 * per * (h + 1), :].rearrange("(t p) w -> p t w", p=128),
            )

    # diagonal d[i] = x[i,i] -> D[:, gi, t] holds d[128t + r']
    if "ddma" in ABLATE:
        nc.vector.memset(gt["D"][:, gi, :], 0.0)
    else:
        nc.sync.dma_start(
            gt["D"][:, gi, :],
            x_d[g].rearrange("h w -> (h w)")[::W + 1].rearrange("(t p) -> p t", p=128),
        )

    # B2[:, gi, t, s] = sum_w x[128t+r', 128s+w]
    if "b2" not in ABLATE:
        nc.vector.tensor_reduce(
            gt["B2"][:, gi].rearrange("p t s -> p (t s)"),
            X.rearrange("p t (s w) -> p (t s) w", w=128),
            axis=AX.X, op=AL.add,
        )
    else:
        nc.vector.memset(gt["B2"][:, gi], 0.0)

    # masked row partials: Pool affine_select (iota-predicate triangular mask),
    # then one DVE segmented reduce -> RB[:, gi, 0:4]=RLp, 4:8=RAp
    def strided(base_ap, free_dims):
        return dataclasses.replace(base_ap, ap=[base_ap.ap[0]] + free_dims)

    MLMA = spool.tile([128, 16, 128], F32, tag="mlma", name="MLMA")
    diag_g = strided(X[:, 0, 0:128], [[640, 4], [1, 128]])    # block (t,t)
    anti_g = strided(X[:, 0, 384:512], [[384, 4], [1, 128]])  # block (t,3-t)
    if "mlma" not in ABLATE:
        nc.gpsimd.affine_select(MLMA[:, 0:4, :], diag_g, pattern=[[0, 4], [-1, 128]],
                                compare_op=AL.is_ge, fill=0.0, base=0, channel_multiplier=1)
        nc.gpsimd.affine_select(MLMA[:, 4:8, :], anti_g, pattern=[[0, 4], [-1, 128]],
                                compare_op=AL.is_ge, fill=0.0, base=127, channel_multiplier=-1)
    else:
        nc.gpsimd.memset(MLMA[:, 0:8, :], 0.0)

    # PE transposes of diag blocks (u,u) and anti blocks (3-u,u)
    TDTA = ppool.tile([128, 8, 128], F32, tag="tdta", name="tdta")
    TT = epool.tile([128, 8, 128], F32, tag="tt", name="tt")
    if "tdta" in ABLATE:
        nc.scalar.memzero(TT[:])
    else:
        for u in range(4):
            nc.tensor.transpose(TDTA[:, u, :], X[:, u, 128 * u:128 * (u + 1)], ident[:])
            nc.tensor.transpose(TDTA[:, 4 + u, :], X[:, 3 - u, 128 * u:128 * (u + 1)], ident[:])
        nc.scalar.copy(TT[:], TDTA[:])


    # column sums via PE: block (s,u) col sums = X-block.T @ ones -> [128, 1]
    # lands in partition layout directly; LDW of the block is the real cost,
    # the N=1 fp32 stream is free. Accumulated into a per-group PSUM tile.
    if "cp" not in ABLATE:
        for s in range(4):
            for u in range(4):
                nc.tensor.matmul(gt["CG"][:, gi, s, u:u + 1], X[:, s, 128 * u:128 * (u + 1)],
                                 ones1[:], start=True, stop=True, skip_group_check=True)
    state[g] = (TT, MLMA)


def _emit_image_b(nc, tc, pools, consts, gi, g, gt, state):
    """Phase B (one image behind A, so PE/ACT/DVE never stall on each other):
    column re-layout transposes + masked column partials."""
    (xpool, epool, spool, ppool, pcg) = pools
    ident = consts["ident"]
    TT, MLMA = state.pop(g)

    if "mc" not in ABLATE:
        nc.gpsimd.affine_select(MLMA[:, 8:12, :], TT[:, 0:4, :], pattern=[[0, 4], [-1, 128]],
                                compare_op=AL.is_ge, fill=0.0, base=0, channel_multiplier=1)
        nc.gpsimd.affine_select(MLMA[:, 12:16, :], TT[:, 4:8, :], pattern=[[0, 4], [-1, 128]],
                                compare_op=AL.is_ge, fill=0.0, base=127, channel_multiplier=-1)
    else:
        nc.gpsimd.memset(MLMA[:, 8:16, :], 0.0)
    if "rbred" not in ABLATE:
        nc.vector.tensor_reduce(gt["RB"][:, gi, :], MLMA[:], axis=AX.X, op=AL.add)
    else:
        nc.vector.memset(gt["RB"][:, gi, :], 0.0)




def _emit_group_combine(nc, tc, gpool, prev_pool, consts, out_d, grp, gt):
    """Batched combine over G images: all ops on [128, (G,4)] tiles."""
    rev, fin = consts["rev"], consts["fin"]
    B2 = gt["B2"]
    RLp, RAp = gt["RB"][:, :, 0:4], gt["RB"][:, :, 4:8]
    CLp, CAp = gt["RB"][:, :, 8:12], gt["RB"][:, :, 12:16]
    D, S = gt["D"], gt["S"]

    def T(tag):
        return gpool.tile([128, G, 4], F32, tag=tag, name=tag)

    tt = nc.vector.tensor_tensor

    # evacuate the PSUM column-sum group tile, then coarse column prefixes
    CCg = gpool.tile([128, G, 4, 4], F32, tag="CCg", name="CCg")
    nc.scalar.copy(CCg[:], gt["CG"][:])
    CT = gpool.tile([128, G, 4], F32, tag="CT", name="CT")
    tt(CT[:], CCg[:, :, 0, :], CCg[:, :, 1, :], op=AL.add)
    tt(CT[:], CT[:], CCg[:, :, 2, :], op=AL.add)
    tt(CT[:], CT[:], CCg[:, :, 3, :], op=AL.add)
    CLc = gpool.tile([128, G, 4], F32, tag="CLc", name="CLc")
    nc.vector.tensor_copy(CLc[:, :, 1:4], CCg[:, :, 0, 1:4])
    nc.vector.memset(CLc[:, :, 0:1], 0.0)
    tt(CLc[:, :, 2:4], CLc[:, :, 2:4], CCg[:, :, 1, 2:4], op=AL.add)
    tt(CLc[:, :, 3:4], CLc[:, :, 3:4], CCg[:, :, 2, 3:4], op=AL.add)
    CAc = gpool.tile([128, G, 4], F32, tag="CAc", name="CAc")
    nc.vector.tensor_copy(CAc[:, :, 0:3], CCg[:, :, 0, 0:3])
    nc.vector.memset(CAc[:, :, 3:4], 0.0)
    tt(CAc[:, :, 0:2], CAc[:, :, 0:2], CCg[:, :, 1, 0:2], op=AL.add)
    tt(CAc[:, :, 0:1], CAc[:, :, 0:1], CCg[:, :, 2, 0:1], op=AL.add)

    # coarse row prefixes from B2
    RT = T("RT")
    tt(RT[:], B2[:, :, :, 0], B2[:, :, :, 1], op=AL.add)
    tt(RT[:], RT[:], B2[:, :, :, 2], op=AL.add)
    tt(RT[:], RT[:], B2[:, :, :, 3], op=AL.add)

    PS = T("PS")
    nc.vector.tensor_copy(PS[:, :, 1:4], B2[:, :, 1:4, 0])
    nc.vector.memset(PS[:, :, 0:1], 0.0)
    tt(PS[:, :, 2:4], PS[:, :, 2:4], B2[:, :, 2:4, 1], op=AL.add)
    tt(PS[:, :, 3:4], PS[:, :, 3:4], B2[:, :, 3:4, 2], op=AL.add)

    PA = T("PA")
    nc.vector.tensor_copy(PA[:, :, 0:3], B2[:, :, 0:3, 0])
    nc.vector.memset(PA[:, :, 3:4], 0.0)
    tt(PA[:, :, 0:2], PA[:, :, 0:2], B2[:, :, 0:2, 1], op=AL.add)
    tt(PA[:, :, 0:1], PA[:, :, 0:1], B2[:, :, 0:1, 2], op=AL.add)

    RL = T("RL")
    tt(RL[:], RLp, PS[:], op=AL.add)
    # RA -> S[:,:,0,:]
    tt(S[:, :, 0, :], RAp, PA[:], op=AL.add)

    CL = T("CL")
    tt(CL[:], CLp, CLc[:], op=AL.add)
    # CA -> S[:,:,1,:]
    tt(S[:, :, 1, :], CAp, CAc[:], op=AL.add)

    # U = RT - RL + d + CT - CL  -> S[:,:,2,:]
    U = S[:, :, 2, :]
    tt(U, RT[:], RL[:], op=AL.subtract)
    tt(U, U, D[:], op=AL.add)
    tt(U, U, CT[:], op=AL.add)
    tt(U, U, CL[:], op=AL.subtract)

    # partition-reversal of S via PE
    Rp = prev_pool.tile([128, G, 3, 4], F32, tag="rp", name="rp")
    nc.tensor.matmul(Rp[:], rev[:], S[:], start=True, stop=True, skip_group_check=True)

    # rings
    r_tl = T("rtl")
    tt(r_tl[:], RL[:], CL[:], op=AL.add)
    tt(r_tl[:], r_tl[:], D[:], op=AL.subtract)
    r_tr = T("rtr")
    tt(r_tr[:], RT[:], S[:, :, 0, :], op=AL.subtract)
    tt(r_tr[:], r_tr[:], Rp[:, :, 1, ::-1], op=AL.add)
    r_bl = T("rbl")
    tt(r_bl[:], Rp[:, :, 0, ::-1], CT[:], op=AL.add)
    tt(r_bl[:], r_bl[:], S[:, :, 1, :], op=AL.subtract)

    # weighted sum + bias + relu;  fin[:, q] q=0..3 weights/denom, q=4 bias
    acc = T("acc")
    m2 = T("m2")
    tt(acc[:], r_tl[:], fin[:, 0], op=AL.mult)
    tt(m2[:], r_tr[:], fin[:, 1], op=AL.mult)
    tt(acc[:], acc[:], m2[:], op=AL.add)
    tt(m2[:], r_bl[:], fin[:, 2], op=AL.mult)
    tt(acc[:], acc[:], m2[:], op=AL.add)
    tt(m2[:], Rp[:, :, 2, ::-1], fin[:, 3], op=AL.mult)
    tt(acc[:], acc[:], m2[:], op=AL.add)
    tt(acc[:], acc[:], fin[:, 4], op=AL.add)
    outsb = T("outsb")
    nc.vector.tensor_scalar_max(outsb[:], acc[:], 0.0)

    for gi in range(G):
        g = grp * G + gi
        nc.sync.dma_start(
            out_d[g].rearrange("(t p) -> p t", p=128),
            outsb[:, gi, :],
        )


def build_bass():
    nc = bass.Bass(trn_type="TRN2")
    x_d = nc.dram_tensor("x", [BPC, H, W], F32, kind="ExternalInput")
    names = {
        "ident": [128, 128],
        "ones1": [128, 1], "rev": [128, 128], "fin": [128, 5, G, 4],
    }
    dts = {k: nc.dram_tensor(k, shp, F32, kind="ExternalInput") for k, shp in names.items()}
    out_d = nc.dram_tensor("out", [BPC, W], F32, kind="ExternalOutput")

    with ExitStack() as ctx:
        tc = ctx.enter_context(tile.TileContext(nc))
        cpool = ctx.enter_context(tc.tile_pool(name="consts", bufs=1))
        xpool = ctx.enter_context(tc.tile_pool(name="xin", bufs=int(os.environ.get("XBUFS", "3"))))
        epool = ctx.enter_context(tc.tile_pool(name="evac", bufs=2))
        spool = ctx.enter_context(tc.tile_pool(name="scr", bufs=int(os.environ.get("SBUFS", "2"))))
        gpool = ctx.enter_context(tc.tile_pool(name="grp", bufs=2))
        ppool = ctx.enter_context(tc.tile_pool(name="ptd", bufs=2, space="PSUM"))
        pcg = ctx.enter_context(tc.tile_pool(name="pcg", bufs=2, space="PSUM"))
        prev_pool = ctx.enter_context(tc.tile_pool(name="prev", bufs=1, space="PSUM"))

        consts = {}
        for k in names:
            t = cpool.tile(names[k], F32, tag=f"c_{k}", name=f"c_{k}")
            nc.sync.dma_start(t[:], dts[k][:])
            consts[k] = t

        # PE warmup: observe the const-DMA queue tick on PE's clock so real
        # transposes carry only their X-DMA wait (transpose LW has 1 wait slot)
        warm = prev_pool.tile([128, 128], F32, tag="rp", name="warm")
        nc.tensor.transpose(warm[:], consts["ident"][:], consts["ident"][:])


        pools = (xpool, epool, spool, ppool, pcg)
        state = {}
        gts = {}

        def make_gt():
            return {
                "B2": gpool.tile([128, G, 4, 4], F32, tag="B2", name="B2"),
                "RB": gpool.tile([128, G, 16], F32, tag="RB", name="RB"),
                "CG": pcg.tile([128, G, 4, 4], F32, tag="CG", name="CG"),
                "D": gpool.tile([128, G, 4], F32, tag="D", name="D"),
                "S": gpool.tile([128, G, 3, 4], F32, tag="S", name="S"),
            }

        for g in range(BPC):
            grp = g // G
            if g % G == 0:
                gts[grp] = make_gt()
            _emit_image_a(nc, tc, pools, consts, x_d, g % G, g, gts[grp], state)
            if g >= 1:
                pg = g - 1
                _emit_image_b(nc, tc, pools, consts, pg % G, pg, gts[pg // G], state)
                if pg % G == G - 1:
                    _emit_group_combine(nc, tc, gpool, prev_pool, consts, out_d,
                                        pg // G, gts.pop(pg // G))
        _emit_image_b(nc, tc, pools, consts, (BPC - 1) % G, BPC - 1,
                      gts[(BPC - 1) // G], state)
        _emit_group_combine(nc, tc, gpool, prev_pool, consts, out_d,
                            NG - 1, gts.pop(NG - 1))
    return nc


def _host_consts(weights, biases):
    r = np.arange(128)
    maskl = (r[None, :] <= r[:, None]).astype(np.float32)          # w <= r
    maska = (r[None, :] <= 127 - r[:, None]).astype(np.float32)    # w <= 127-r
    ident = np.eye(128, dtype=np.float32)
    ones1 = np.ones((128, 1), np.float32)
    rev = np.zeros((128, 128), np.float32)
    rev[r, 127 - r] = 1.0                                           # REV[k,m]=[k==127-m]
    i = np.arange(512)
    den = (2 * i + 1).astype(np.float32)
    fin = np.zeros((128, 5, G, 4), np.float32)
    wq = weights[:, :, 0] / den[:, None]                            # [512, 4]
    for t in range(4):
        for q in range(4):
            fin[:, q, :, t] = wq[128 * t:128 * (t + 1), q][:, None]
        fin[:, 4, :, t] = biases[128 * t:128 * (t + 1), 0][:, None]
    return {"ident": ident,
            "ones1": ones1, "rev": rev, "fin": fin}


def split_waits(nc, max_waits=1):
    """This walrus build rejects instructions with more than one sync wait.
    Hoist extra waits onto standalone NoOps on the same engine."""
    for fn in nc.m.functions:
        for blk in fn.blocks:
            new_insts = []
            for inst in blk.instructions:
                si = inst.sync_info
                ow = list(si.on_wait) if si is not None and si.on_wait else []
                if len(ow) > max_waits:
                    for k, w in enumerate(ow[max_waits:]):
                        nop = mybir.InstNoOp(
                            name=f"{inst.name}-w{k}", ins=[], outs=[],
                            sync_info=mybir.SyncInfo(on_wait=[w], on_update=[]),
                        )
                        nop.engine = inst.engine
                        new_insts.append(nop)
                    si.on_wait = ow[:max_waits]
                    inst.sync_info = si
                new_insts.append(inst)
            blk.instructions = new_insts
    return nc


_CACHE = {}


def kernel(x, weights, biases, trace=False, **run_kwargs):
    x = np.ascontiguousarray(x, dtype=np.float32).reshape(B_TOT, H, W)
    weights = np.asarray(weights, dtype=np.float32)
    biases = np.asarray(biases, dtype=np.float32)
    if "nc" not in _CACHE:
        _CACHE["nc"] = split_waits(build_bass())
    nc = _CACHE["nc"]
    consts = _host_consts(weights, biases)
    in_maps = []
    for c in range(NCORES):
        m = {"x": np.ascontiguousarray(x[c * BPC:(c + 1) * BPC])}
        m.update(consts)
        in_maps.append(m)
    res = run_bass_kernel_spmd(nc, in_maps, core_ids=list(range(NCORES)),
                               trace=trace, **run_kwargs)
    out = np.concatenate([r["out"] for r in res.results], axis=0)
    _CACHE["last_result"] = res
    return out



# revision 8
# speedup vs baseline: 2.5035x; 2.5035x over previous
"""Trainium2 Bass kernel for nn_DiagonalKernelAverageLinear (v2).

out[b,i] = relu( sum_q w[i,q] * ring_q[b,i] / (2i+1) + bias[i] )

d-free ring decomposition (verified vs reference in fp64):
  ring_tl = RL' + CL
  ring_tr = RT - RA + rev(CA)
  ring_bl = rev(RA) + CT - CA
  ring_br = rev(RT - RL' + CT - CL)
with per-image length-512 vectors (r, c are row/col index; rev(v)[i]=v[511-i]):
  RL'[r] = sum_{w<r} x[r,w]      RA[r] = sum_{w<=511-r} x[r,w]   RT = row sums
  CL[c]  = sum_{h<=c} x[h,c]     CA[c] = sum_{h<=511-c} x[h,c]   CT = col sums

x is cast to fp16 and permuted to [g, p, t, w] on the HOST so each partition's
per-image slice is one contiguous 4KB DMA run (2x less HBM traffic, 4KB
packets instead of 2KB). No diagonal DMA at all (strict RL' removes d).

Engine split per image:
  - Pool: 3 affine_selects (column-fine masks on the original diag/anti
    blocks, row-diag strict mask).
  - DVE: row-anti mask via tensor_tensor with a mask constant (fp16 2x),
    then one merged fp16 tt-tree (128->64->32->16 halving) + final small
    reduce producing ALL free-axis sums at once: 16 B2 block sums + 4 RL'
    fine + 4 RA fine. Plus most of the per-group combine.
  - PE (fp16: LDW is 4x cheaper than fp32): 24 block-LDW matmuls with a
    ones vector contract the partition axis: 16 coarse column block sums
    (CG) + 8 fine column partials from the Pool-masked copies (CF). No PE
    transposes of X. Per group: one reversal matmul.
  - ACT: per-group PSUM evacuations + a slice of B2 if rebalancing.
"""

import dataclasses
import os
import numpy as np
from contextlib import ExitStack

import concourse.bass as bass
import concourse.mybir as mybir
import concourse.tile as tile
from concourse.bass_utils import run_bass_kernel_spmd

F32 = mybir.dt.float32
F16 = mybir.dt.float16
AL = mybir.AluOpType
AX = mybir.AxisListType

NCORES = 8
B_TOT, H, W = 256, 512, 512
BPC = B_TOT // NCORES          # 32 images per core
G = int(os.environ.get("KG", "16"))  # combine-group size
NG = BPC // G


def strided(base_ap, free_dims):
    """Replace the free dims of an AP with explicit [stride, count] pairs
    (strides in elements)."""
    return dataclasses.replace(base_ap, ap=[base_ap.ap[0]] + free_dims)


def _emit_image(nc, tc, pools, consts, x_d, gi, g, gt):
    (xpool, mpool, tpool) = pools
    ones1 = consts["ones1"]
    M2rep = consts["m2rep"]

    X = xpool.tile([128, 4, 512], F16, tag="X", name="X")
    nc.sync.dma_start(X[:], x_d[g])

    # strided views of the 4 diagonal blocks (t,t) and 4 anti blocks (t,3-t)
    diag_g = strided(X[:, 0, 0:128], [[640, 4], [1, 128]])
    anti_g = strided(X[:, 0, 384:512], [[384, 4], [1, 128]])

    MC = mpool.tile([128, 8, 128], F16, tag="MC", name="MC")
    MR = mpool.tile([128, 8, 128], F16, tag="MR", name="MR")

    # Pool: column-fine masks on original blocks (partition = row-in-block)
    #   CLf source: keep (p, c) where c - p >= 0   (h_l <= c_l)
    nc.gpsimd.affine_select(MC[:, 0:4, :], diag_g, pattern=[[0, 4], [1, 128]],
                            compare_op=AL.is_ge, fill=0.0, base=0, channel_multiplier=-1)
    #   CAf source: keep where 127 - p - c >= 0   (h_l <= 127 - c_l)
    nc.gpsimd.affine_select(MC[:, 4:8, :], anti_g, pattern=[[0, 4], [-1, 128]],
                            compare_op=AL.is_ge, fill=0.0, base=127, channel_multiplier=-1)
    #   RL' fine (strict): keep (p, w) where p - w - 1 >= 0   (w < p)
    nc.gpsimd.affine_select(MR[:, 0:4, :], diag_g, pattern=[[0, 4], [-1, 128]],
                            compare_op=AL.is_ge, fill=0.0, base=-1, channel_multiplier=1)
    # DVE: RA fine mask (w_l <= 127 - p) via constant-mask multiply (fp16 2x)
    nc.vector.tensor_tensor(MR[:, 4:8, :], anti_g, M2rep[:], op=AL.mult)

    # PE block-LDW matmuls (partition-axis sums), all land at partition base 0:
    #   coarse col sums of all 16 blocks: CG[:, gi, s, u] = colsum block (s,u)
    for s in range(4):
        for u in range(4):
            nc.tensor.matmul(gt["CG"][:, gi, s, u:u + 1], X[:, s, 128 * u:128 * (u + 1)],
                             ones1[:], start=True, stop=True, skip_group_check=True)
    #   fine col partials: CF[:, gi, 0, u] = CLf(u), CF[:, gi, 1, u] = CAf(u)
    for u in range(4):
        nc.tensor.matmul(gt["CF"][:, gi, 0, u:u + 1], MC[:, u, :],
                         ones1[:], start=True, stop=True, skip_group_check=True)
    # anti slice t covers column block 3-t
    for u in range(4):
        nc.tensor.matmul(gt["CF"][:, gi, 1, 3 - u:4 - u], MC[:, 4 + u, :],
                         ones1[:], start=True, stop=True, skip_group_check=True)

    # DVE merged tt-tree: slices 0:16 = X blocks (B2), 16:20 = RL'f, 20:24 = RAf
    Xv = X[:].rearrange("p t (s w) -> p (t s) w", w=128)
    T1 = tpool.tile([128, 24, 64], F16, tag="T1", name="T1")
    T2 = tpool.tile([128, 24, 32], F16, tag="T2", name="T2")
    T3 = tpool.tile([128, 24, 16], F16, tag="T3", name="T3")
    tt = nc.vector.tensor_tensor
    tt(T1[:, 0:16, :], Xv[:, :, 0:64], Xv[:, :, 64:128], op=AL.add)
    tt(T1[:, 16:24, :], MR[:, :, 0:64], MR[:, :, 64:128], op=AL.add)
    tt(T2[:], T1[:, :, 0:32], T1[:, :, 32:64], op=AL.add)
    tt(T3[:], T2[:, :, 0:16], T2[:, :, 16:32], op=AL.add)
    nc.vector.tensor_reduce(gt["RB"][:, gi, :], T3[:], axis=AX.X, op=AL.add)


def _emit_group_combine(nc, tc, gpool, prev_pool, consts, out_d, grp, gt):
    """Batched combine over G images: all ops on [128, (G,4)] tiles."""
    rev, fin = consts["rev"], consts["fin"]

    # evacuate PSUM column sums to SBUF
    CCg = gpool.tile([128, G, 4, 4], F32, tag="CCg", name="CCg")
    CFs = gpool.tile([128, G, 2, 4], F32, tag="CFs", name="CFs")
    nc.scalar.copy(CCg[:], gt["CG"][:])
    nc.scalar.copy(CFs[:], gt["CF"][:])

    B2 = gt["RB"][:, :, 0:16].rearrange("p g (t s) -> p g t s", s=4)
    RLf, RAf = gt["RB"][:, :, 16:20], gt["RB"][:, :, 20:24]

    def T(tag):
        return gpool.tile([128, G, 4], F32, tag=tag, name=tag)

    tt = nc.vector.tensor_tensor
    ttp = nc.gpsimd.tensor_tensor

    # ---- Pool subtree: row coarse prefixes + RL/RA assembly ----
    RT = T("RT")
    ttp(RT[:], B2[:, :, :, 0], B2[:, :, :, 1], op=AL.add)
    ttp(RT[:], RT[:], B2[:, :, :, 2], op=AL.add)
    ttp(RT[:], RT[:], B2[:, :, :, 3], op=AL.add)

    PS = T("PS")
    nc.gpsimd.memset(PS[:, :, 0:1], 0.0)
    nc.gpsimd.tensor_copy(PS[:, :, 1:4], B2[:, :, 1:4, 0])
    ttp(PS[:, :, 2:4], PS[:, :, 2:4], B2[:, :, 2:4, 1], op=AL.add)
    ttp(PS[:, :, 3:4], PS[:, :, 3:4], B2[:, :, 3:4, 2], op=AL.add)

    PA = T("PA")
    nc.gpsimd.memset(PA[:, :, 3:4], 0.0)
    nc.gpsimd.tensor_copy(PA[:, :, 0:3], B2[:, :, 0:3, 0])
    ttp(PA[:, :, 0:2], PA[:, :, 0:2], B2[:, :, 0:2, 1], op=AL.add)
    ttp(PA[:, :, 0:1], PA[:, :, 0:1], B2[:, :, 0:1, 2], op=AL.add)

    RL = T("RL")
    ttp(RL[:], RLf, PS[:], op=AL.add)
    S = gt["S"]
    ttp(S[:, :, 0, :], RAf, PA[:], op=AL.add)           # RA

    # ---- DVE subtree: column assembly ----
    CT = T("CT")
    tt(CT[:], CCg[:, :, 0, :], CCg[:, :, 1, :], op=AL.add)
    tt(CT[:], CT[:], CCg[:, :, 2, :], op=AL.add)
    tt(CT[:], CT[:], CCg[:, :, 3, :], op=AL.add)

    CLc = T("CLc")
    nc.vector.memset(CLc[:, :, 0:1], 0.0)
    nc.vector.tensor_copy(CLc[:, :, 1:4], CCg[:, :, 0, 1:4])
    tt(CLc[:, :, 2:4], CLc[:, :, 2:4], CCg[:, :, 1, 2:4], op=AL.add)
    tt(CLc[:, :, 3:4], CLc[:, :, 3:4], CCg[:, :, 2, 3:4], op=AL.add)

    CAc = T("CAc")
    nc.vector.memset(CAc[:, :, 3:4], 0.0)
    nc.vector.tensor_copy(CAc[:, :, 0:3], CCg[:, :, 0, 0:3])
    tt(CAc[:, :, 0:2], CAc[:, :, 0:2], CCg[:, :, 1, 0:2], op=AL.add)
    tt(CAc[:, :, 0:1], CAc[:, :, 0:1], CCg[:, :, 2, 0:1], op=AL.add)

    CL = T("CL")
    tt(CL[:], CFs[:, :, 0, :], CLc[:], op=AL.add)
    tt(S[:, :, 1, :], CFs[:, :, 1, :], CAc[:], op=AL.add)   # CA

    # U = RT - RL + CT - CL  -> S[:,:,2,:]
    U = S[:, :, 2, :]
    tt(U, RT[:], RL[:], op=AL.subtract)
    tt(U, U, CT[:], op=AL.add)
    tt(U, U, CL[:], op=AL.subtract)

    # partition-reversal of S via PE
    Rp = prev_pool.tile([128, G, 3, 4], F32, tag="rp", name="rp")
    nc.tensor.matmul(Rp[:], rev[:], S[:], start=True, stop=True, skip_group_check=True)

    # rings
    r_tl = T("rtl")
    tt(r_tl[:], RL[:], CL[:], op=AL.add)
    r_tr = T("rtr")
    tt(r_tr[:], RT[:], S[:, :, 0, :], op=AL.subtract)
    tt(r_tr[:], r_tr[:], Rp[:, :, 1, ::-1], op=AL.add)
    r_bl = T("rbl")
    tt(r_bl[:], Rp[:, :, 0, ::-1], CT[:], op=AL.add)
    tt(r_bl[:], r_bl[:], S[:, :, 1, :], op=AL.subtract)

    # weighted sum + bias + relu;  fin[:, q] q=0..3 weights/denom, q=4 bias
    acc = T("acc")
    m2 = T("m2")
    tt(acc[:], r_tl[:], fin[:, 0], op=AL.mult)
    tt(m2[:], r_tr[:], fin[:, 1], op=AL.mult)
    tt(acc[:], acc[:], m2[:], op=AL.add)
    tt(m2[:], r_bl[:], fin[:, 2], op=AL.mult)
    tt(acc[:], acc[:], m2[:], op=AL.add)
    tt(m2[:], Rp[:, :, 2, ::-1], fin[:, 3], op=AL.mult)
    tt(acc[:], acc[:], m2[:], op=AL.add)
    tt(acc[:], acc[:], fin[:, 4], op=AL.add)
    outsb = T("outsb")
    nc.vector.tensor_scalar_max(outsb[:], acc[:], 0.0)

    dbg = os.environ.get("KDBG", "")
    if dbg:
        src = {
            "RT": RT, "PS": PS, "PA": PA, "RL": RL, "CT": CT, "CLc": CLc,
            "CAc": CAc, "CL": CL, "rtl": r_tl, "rtr": r_tr, "rbl": r_bl,
            "acc": acc,
        }.get(dbg)
        if src is not None:
            nc.vector.tensor_copy(outsb[:], src[:])
        elif dbg == "RA":
            nc.vector.tensor_copy(outsb[:], S[:, :, 0, :])
        elif dbg == "CA":
            nc.vector.tensor_copy(outsb[:], S[:, :, 1, :])
        elif dbg == "U":
            nc.vector.tensor_copy(outsb[:], S[:, :, 2, :])
        elif dbg == "RLf":
            nc.vector.tensor_copy(outsb[:], RLf)
        elif dbg == "RAf":
            nc.vector.tensor_copy(outsb[:], RAf)
        elif dbg == "CLf":
            nc.vector.tensor_copy(outsb[:], CFs[:, :, 0, :])
        elif dbg == "CAf":
            nc.vector.tensor_copy(outsb[:], CFs[:, :, 1, :])
        elif dbg == "rbr":
            nc.vector.tensor_copy(outsb[:], Rp[:, :, 2, ::-1])

    nc.sync.dma_start(out_d[grp], outsb[:])


def build_bass():
    nc = bass.Bass(trn_type="TRN2")
    x_d = nc.dram_tensor("x", [BPC, 128, 4, 512], F16, kind="ExternalInput")
    names = {
        "ones1": ([128, 1], F16),
        "m2rep": ([128, 4, 128], F16),
        "rev": ([128, 128], F32),
        "fin": ([128, 5, G, 4], F32),
    }
    dts = {k: nc.dram_tensor(k, shp, dt, kind="ExternalInput")
           for k, (shp, dt) in names.items()}
    out_d = nc.dram_tensor("out", [NG, 128, G, 4], F32, kind="ExternalOutput")

    with ExitStack() as ctx:
        tc = ctx.enter_context(tile.TileContext(nc))
        cpool = ctx.enter_context(tc.tile_pool(name="consts", bufs=1))
        xpool = ctx.enter_context(tc.tile_pool(name="xin", bufs=int(os.environ.get("XBUFS", "3"))))
        mpool = ctx.enter_context(tc.tile_pool(name="msk", bufs=2))
        tpool = ctx.enter_context(tc.tile_pool(name="tree", bufs=2))
        gpool = ctx.enter_context(tc.tile_pool(name="grp", bufs=2))
        psum_cs = ctx.enter_context(tc.tile_pool(name="pcs", bufs=2, space="PSUM"))
        prev_pool = ctx.enter_context(tc.tile_pool(name="prev", bufs=1, space="PSUM"))

        consts = {}
        for k in names:
            t = cpool.tile(list(names[k][0]), names[k][1], tag=f"c_{k}", name=f"c_{k}")
            nc.sync.dma_start(t[:], dts[k][:])
            consts[k] = t

        # PE warmup on the const-DMA queue tick
        warm = prev_pool.tile([128, 128], F32, tag="rp", name="warm")
        nc.tensor.transpose(warm[:], consts["rev"][:], consts["rev"][:])

        pools = (xpool, mpool, tpool)
        gts = {}

        def make_gt():
            return {
                "CG": psum_cs.tile([128, G, 4, 4], F32, tag="CG", name="CG"),
                "CF": psum_cs.tile([128, G, 2, 4], F32, tag="CF", name="CF"),
                "RB": gpool.tile([128, G, 24], F32, tag="RB", name="RB"),
                "S": gpool.tile([128, G, 3, 4], F32, tag="S", name="S"),
            }

        for g in range(BPC):
            grp = g // G
            if g % G == 0:
                gts[grp] = make_gt()
            _emit_image(nc, tc, pools, consts, x_d, g % G, g, gts[grp])
            if g % G == G - 1:
                _emit_group_combine(nc, tc, gpool, prev_pool, consts,
                                    out_d, grp, gts.pop(grp))
    return nc


def _host_consts(weights, biases):
    r = np.arange(128)
    ones1 = np.ones((128, 1), np.float16)
    m2 = (r[None, :] <= 127 - r[:, None]).astype(np.float16)   # w <= 127-p
    m2rep = np.broadcast_to(m2[:, None, :], (128, 4, 128)).copy()
    rev = np.zeros((128, 128), np.float32)
    rev[r, 127 - r] = 1.0
    i = np.arange(512)
    den = (2 * i + 1).astype(np.float32)
    fin = np.zeros((128, 5, G, 4), np.float32)
    wq = weights[:, :, 0] / den[:, None]                        # [512, 4]
    for t in range(4):
        for q in range(4):
            fin[:, q, :, t] = wq[128 * t:128 * (t + 1), q][:, None]
        fin[:, 4, :, t] = biases[128 * t:128 * (t + 1), 0][:, None]
    return {"ones1": ones1, "m2rep": m2rep, "rev": rev, "fin": fin}


def split_waits(nc, max_waits=1):
    """Hoist extra sync waits onto standalone NoOps on the same engine."""
    for fn in nc.m.functions:
        for blk in fn.blocks:
            new_insts = []
            for inst in blk.instructions:
                si = inst.sync_info
                ow = list(si.on_wait) if si is not None and si.on_wait else []
                if len(ow) > max_waits:
                    for k, w in enumerate(ow[max_waits:]):
                        nop = mybir.InstNoOp(
                            name=f"{inst.name}-w{k}", ins=[], outs=[],
                            sync_info=mybir.SyncInfo(on_wait=[w], on_update=[]),
                        )
                        nop.engine = inst.engine
                        new_insts.append(nop)
                    si.on_wait = ow[:max_waits]
                    inst.sync_info = si
                new_insts.append(inst)
            blk.instructions = new_insts
    return nc


_CACHE = {}


def kernel(x, weights, biases, trace=False, **run_kwargs):
    x = np.ascontiguousarray(x, dtype=np.float32).reshape(B_TOT, H, W)
    weights = np.asarray(weights, dtype=np.float32)
    biases = np.asarray(biases, dtype=np.float32)
    if "nc" not in _CACHE:
        _CACHE["nc"] = split_waits(build_bass())
    nc = _CACHE["nc"]
    consts = _host_consts(weights, biases)
    # host-side: fp16 cast + [g, 128t+p, w] -> [g, p, t, w] permute
    x16 = x.astype(np.float16).reshape(B_TOT, 4, 128, 512).transpose(0, 2, 1, 3)
    in_maps = []
    for c in range(NCORES):
        m = {"x": np.ascontiguousarray(x16[c * BPC:(c + 1) * BPC])}
        m.update(consts)
        in_maps.append(m)
    res = run_bass_kernel_spmd(nc, in_maps, core_ids=list(range(NCORES)),
                               trace=trace, **run_kwargs)
    # out [NG, 128, G, 4] -> [BPC, 512]: out[grp*G+gi, 128t+p] = o[grp, p, gi, t]
    outs = []
    for r in res.results:
        o = r["out"].reshape(NG, 128, G, 4)
        outs.append(o.transpose(0, 2, 3, 1).reshape(BPC, 512))
    out = np.concatenate(outs, axis=0)
    _CACHE["last_result"] = res
    return out
